# revision 2
# baseline (speedup 1.0000x reference)
"""Trainium2 Bass kernel for nn_Interpolator: pilot-to-subcarrier linear
interpolation with learned per-subcarrier weights.

Math: out[b, t] = alpha[t] * Hp[b, right[t]] + beta[t] * Hp[b, left[t]]
where Hp = [H, extrapolated last column]. The op is linear in H, so it
collapses to out = H @ W with a sparse W [256, 4096] built on the host
from (pilot_loc, alpha, beta); the extrapolation column folds into W's
last two rows.

Compression: output column t only depends on column t of W, so columns
of W that are identical produce identical output columns. The device
computes out_u = H @ Wu for the U *unique* columns of W only, and the
host scatters out_u's columns back to the full 4096 (pure indexing).
For this module's inputs (constant alpha/beta, stride-16 pilots) U=256,
a 16x cut in device output traffic. Falls back gracefully (same code
path) to any U up to 4096.

Precision: H and Wu are sent in bf16; out_u is stored in bf16. Error is
~2e-3 relative (bf16 rounding in, bf16 rounding out) against a 2e-2
gate. If Wu is not exactly bf16-representable a Wu_lo correction term
is added (for this module's inputs Wu is exact in bf16, so it is not).

Layout: H^T is pre-transposed on the host and packed per core as
hxt [128, 2(comp) x 2(k-half) x 2048(batch)] bf16, so matmul lhsT
slices come straight from SBUF with no on-chip transposes.

Sharding: data-parallel over the batch dim, 2048 rows per core x 8.
"""

import os
import sys

if os.path.isdir("/opt/trn_rl_repo") and "/opt/trn_rl_repo" not in sys.path:
    sys.path.insert(0, "/opt/trn_rl_repo")

import ml_dtypes
import numpy as np

_BF16 = np.dtype(ml_dtypes.bfloat16)

_B, _P, _NFFT = 16384, 256, 4096
_NC = 8
_BS = _B // _NC          # rows per core
_PT = 128                # partition tile (batch rows per tile)
_NBT = _BS // _PT        # batch tiles per core
_CH = 512                # max PSUM chunk width (one bank of fp32)

_cache = {}


def _interp_matrix(pilot_loc, alpha, beta):
    """W [256, 4096] f32 such that out = H @ W reproduces the reference."""
    p = pilot_loc.astype(np.float64) - 1.0  # reference: 1-based -> 0-based
    pp = np.concatenate([p, [float(_NFFT - 1)]])
    t = np.arange(_NFFT)
    left = np.clip(np.searchsorted(pp, t, side="right") - 1, 0, _P - 1)
    right = left + 1
    Wf = np.zeros((_P + 1, _NFFT), np.float64)
    Wf[left, t] += beta.astype(np.float64)
    Wf[right, t] += alpha.astype(np.float64)
    # Hp[:, P] = H[:, P-1] + slope * (NFFT-1 - p[-1]),
    # slope = (H[:, P-1] - H[:, P-2]) / (p[-1] - p[-2])  -> linear in H.
    d = (float(_NFFT - 1) - p[-1]) / (p[-1] - p[-2])
    W = Wf[:_P]
    W[_P - 1] += (1.0 + d) * Wf[_P]
    W[_P - 2] += (-d) * Wf[_P]
    return np.ascontiguousarray(W.astype(np.float32))


def _unique_cols(W):
    """Wu [256, U] = unique columns of W; inv [4096] with W = Wu[:, inv]."""
    uniq, inv = np.unique(W.T, axis=0, return_inverse=True)
    return np.ascontiguousarray(uniq.T), inv.astype(np.int64).ravel()


def _chunk_plan(W_nz, U):
    """Per <=512-col chunk of Wu: list of (col_lo, col_hi, halves).

    Each entry is a maximal run of columns inside the chunk that needs
    the same set of 128-row k-halves (halves with any nonzero in that
    run). Runs with no nonzero anywhere get half 0 (W is zero there, so
    the matmul writes zeros). Full K=128 slices only: every matmul sits
    at PE tile_position (0, 0) and cycle cost is K-independent.
    """
    nch = (U + _CH - 1) // _CH
    plan = []
    for c in range(nch):
        lo, hi = c * _CH, min(U, (c + 1) * _CH)
        need = [np.any(W_nz[128 * h:128 * (h + 1), lo:hi], axis=0)
                for h in (0, 1)]
        key = need[0].astype(np.int8) + 2 * need[1].astype(np.int8)
        runs = []
        s = 0
        for j in range(1, hi - lo + 1):
            if j == hi - lo or key[j] != key[s]:
                halves = [h for h in (0, 1) if need[h][s]] or [0]
                runs.append((lo + s, lo + j, tuple(halves)))
                s = j
        if len(runs) > 8:  # pathological W: one group, all needed halves
            halves = [h for h in (0, 1) if need[h].any()] or [0]
            runs = [(lo, hi, tuple(halves))]
        plan.append(tuple(runs))
    return tuple(plan)


def _bf16_split(x):
    hi = x.astype(_BF16)
    lo = (x - hi.astype(np.float32)).astype(_BF16)
    return hi, lo


def _build_program(U, plan, use_wlo):
    from contextlib import ExitStack

    import concourse.bacc as bacc
    import concourse.bass as bass
    import concourse.mybir as mybir
    import concourse.tile as tile

    f32 = mybir.dt.float32
    bf16 = mybir.dt.bfloat16

    nc = bacc.Bacc("TRN2", target_bir_lowering=False, debug=False,
                   num_devices=_NC)
    # Packed transposed input: [128, comp*4096 + half*2048 + batch].
    h_in = nc.dram_tensor("hxt", [128, 2 * 2 * _BS], bf16,
                          kind="ExternalInput").ap()
    w_in = {"h": nc.dram_tensor("wh", [_P, U], bf16,
                                kind="ExternalInput").ap()}
    if use_wlo:
        w_in["l"] = nc.dram_tensor("wl", [_P, U], bf16,
                                   kind="ExternalInput").ap()
    out = nc.dram_tensor("out", [_BS, 2 * U], bf16,
                         kind="ExternalOutput").ap()

    with tile.TileContext(nc) as tc, ExitStack() as ctx:
        const_pool = ctx.enter_context(tc.tile_pool(name="const", bufs=1))
        out_pool = ctx.enter_context(tc.tile_pool(name="outp", bufs=3))
        ps_mm = ctx.enter_context(tc.tile_pool(name="psm", bufs=4,
                                               space="PSUM"))

        # Weights on the scalar ring, input halves split scalar/gpsimd
        # so the two loads run on parallel rings; stores go on sync.
        w_sb = {}
        for part, wap in w_in.items():
            for h in (0, 1):
                wt = const_pool.tile([128, U], bf16, tag=f"w{part}{h}")
                nc.scalar.dma_start(wt[:], wap[128 * h:128 * (h + 1), :])
                w_sb[(part, h)] = wt

        hx = {}
        for x in (0, 1):          # comp: 0=real, 1=imag
            for h in (0, 1):      # k-half
                t = const_pool.tile([128, _BS], bf16, tag=f"hx{x}{h}")
                eng = nc.scalar if x == 0 else nc.gpsimd
                eng.dma_start(t[:], h_in[:, x * 2 * _BS + h * _BS:
                                         x * 2 * _BS + (h + 1) * _BS])
                hx[(x, h)] = t

        terms = [("h",)] if not use_wlo else [("h",), ("l",)]
        copy_idx = 0
        for bt in range(_NBT):
            ot = out_pool.tile([128, 2 * U], bf16, tag="ot")
            for x in (0, 1):
                for runs in plan:
                    clo, chi = runs[0][0], runs[-1][1]
                    ps = ps_mm.tile([128, chi - clo], f32, tag="ps")
                    for (rlo, rhi, halves) in runs:
                        n_mm = len(halves) * len(terms)
                        j = 0
                        for h in halves:
                            for (wp,) in terms:
                                nc.tensor.matmul(
                                    ps[:, rlo - clo:rhi - clo],
                                    hx[(x, h)][:, bt * 128:(bt + 1) * 128],
                                    w_sb[(wp, h)][:, rlo:rhi],
                                    start=(j == 0),
                                    stop=(j == n_mm - 1),
                                )
                                j += 1
                    dst = ot[:, x * U + clo:x * U + chi]
                    # ~2:1 vector:scalar keeps the two copy engines
                    # balanced (ACT copies are ~2x slower than DVE).
                    if copy_idx % 3 == 2:
                        nc.scalar.copy(dst, ps[:])
                    else:
                        nc.vector.tensor_copy(dst, ps[:])
                    copy_idx += 1
            nc.sync.dma_start(out[bass.ts(bt, 128), :], ot[:])

    nc.compile()
    return nc


def _get_program(U, plan, use_wlo):
    key = (U, plan, use_wlo)
    prog = _cache.get(key)
    if prog is None:
        prog = _build_program(U, plan, use_wlo)
        _cache[key] = prog
    return prog


def _prepare(H_real, H_imag, pilot_loc, alpha, beta):
    """Build (nc, in_maps, (U, inv)) for the spmd run."""
    H_real = np.ascontiguousarray(np.asarray(H_real, dtype=np.float32))
    H_imag = np.ascontiguousarray(np.asarray(H_imag, dtype=np.float32))
    pilot_loc = np.asarray(pilot_loc, dtype=np.float32)
    alpha = np.asarray(alpha, dtype=np.float32)
    beta = np.asarray(beta, dtype=np.float32)

    W = _interp_matrix(pilot_loc, alpha, beta)
    Wu, inv = _unique_cols(W)
    U = Wu.shape[1]
    wu_hi, wu_lo = _bf16_split(Wu)
    use_wlo = bool(np.any(np.asarray(wu_lo) != 0))
    plan = _chunk_plan(Wu != 0.0, U)
    nc = _get_program(U, plan, use_wlo)

    # Transposed bf16 inputs: [256 pilots, 16384 batch] per component.
    hrt = np.ascontiguousarray(H_real.astype(_BF16).T)
    hit = np.ascontiguousarray(H_imag.astype(_BF16).T)

    in_maps = []
    for i in range(_NC):
        cs = slice(i * _BS, (i + 1) * _BS)
        hxt = np.concatenate(
            [hrt[0:128, cs], hrt[128:256, cs],
             hit[0:128, cs], hit[128:256, cs]], axis=1)
        m = {"hxt": np.ascontiguousarray(hxt), "wh": wu_hi}
        if use_wlo:
            m["wl"] = wu_lo
        in_maps.append(m)
    return nc, in_maps, (U, inv)


def _assemble(results, U, inv):
    full = np.concatenate([np.asarray(r["out"]) for r in results], axis=0)
    full = full.astype(np.float32)          # [B, 2U]: [real | imag]
    idx = np.empty(2 * _NFFT, np.int64)     # interleave (r, i) per t
    idx[0::2] = inv
    idx[1::2] = U + inv
    return full[:, idx].reshape(_B, _NFFT, 2)


def kernel(H_real, H_imag, pilot_loc, alpha, beta):
    nc, in_maps, (U, inv) = _prepare(H_real, H_imag, pilot_loc,
                                     alpha, beta)
    from concourse.bass_utils import run_bass_kernel_spmd

    res = run_bass_kernel_spmd(nc, in_maps, list(range(_NC))).results
    return _assemble(res, U, inv)


# revision 6
# speedup vs baseline: 1.2836x; 1.2836x over previous
"""Trainium2 Bass kernel for nn_Interpolator: pilot-to-subcarrier linear
interpolation with learned per-subcarrier weights.

Math: out[b, t] = alpha[t] * Hp[b, right[t]] + beta[t] * Hp[b, left[t]]
where Hp = [H, extrapolated last column]. The op is linear in H, so it
collapses to out = H @ W with a sparse W [256, 4096] built on the host
from (pilot_loc, alpha, beta); the extrapolation column folds into W's
last two rows.

Compression: output column t only depends on column t of W, so columns
of W that are identical produce identical output columns. The device
computes out_u = H @ Wu for the U *unique* columns of W only, and the
host scatters out_u's columns back to the full 4096 (pure indexing).
For this module's inputs (constant alpha/beta, stride-16 pilots) U=256,
a 16x cut in device output traffic. Falls back gracefully (same code
path) to any U up to 4096.

Precision: H and Wu are sent in bf16; out_u is stored in bf16. Error is
~2e-3 relative (bf16 rounding in, bf16 rounding out) against a 2e-2
gate. If Wu is not exactly bf16-representable a Wu_lo correction term
is added (for this module's inputs Wu is exact in bf16, so it is not).

Layout / schedule (from NTFF trace analysis):
- H^T is pre-transposed on the host and packed per core in 2-batch-tile
  "groups" [128, 2(comp) x 2(k-half) x 2(tile) x 128] so matmul lhsT
  slices come straight from SBUF with no on-chip transposes, and the
  8 group loads spread across 5 DMA rings (tensor/gpsimd/vector/
  scalar/sync) - the first matmul only waits for group 0 (~256KB).
- Whole-chunk matmuls: per batch tile, per comp, one N<=512 matmul per
  k-half accumulating into one PSUM bank (zero W regions contribute 0);
  per-matmul fixed cost (~150-200ns) dominates streaming, so fewer,
  wider matmuls win over skipping zero halves.
- One fp32->bf16 cast per PSUM tile, alternating DVE/ACT engines; out
  DMAs alternate sync/gpsimd rings so the write stream keeps pace with
  compute (single ring measured ~130GB/s at 1KB rows).

Sharding: data-parallel over the batch dim, 2048 rows per core x 8.
"""

import os
import sys

if os.path.isdir("/opt/trn_rl_repo") and "/opt/trn_rl_repo" not in sys.path:
    sys.path.insert(0, "/opt/trn_rl_repo")

import ml_dtypes
import numpy as np

_BF16 = np.dtype(ml_dtypes.bfloat16)

_B, _P, _NFFT = 16384, 256, 4096
_NC = 8
_BS = _B // _NC          # rows per core
_PT = 128                # partition tile (batch rows per tile)
_NBT = _BS // _PT        # batch tiles per core
_GT = 2                  # batch tiles per input-load group
_NG = _NBT // _GT        # input-load groups
_CH = 512                # max PSUM chunk width (one bank of fp32)

_cache = {}


def _interp_matrix(pilot_loc, alpha, beta):
    """W [256, 4096] f32 such that out = H @ W reproduces the reference."""
    p = pilot_loc.astype(np.float64) - 1.0  # reference: 1-based -> 0-based
    pp = np.concatenate([p, [float(_NFFT - 1)]])
    t = np.arange(_NFFT)
    left = np.clip(np.searchsorted(pp, t, side="right") - 1, 0, _P - 1)
    right = left + 1
    Wf = np.zeros((_P + 1, _NFFT), np.float64)
    Wf[left, t] += beta.astype(np.float64)
    Wf[right, t] += alpha.astype(np.float64)
    # Hp[:, P] = H[:, P-1] + slope * (NFFT-1 - p[-1]),
    # slope = (H[:, P-1] - H[:, P-2]) / (p[-1] - p[-2])  -> linear in H.
    d = (float(_NFFT - 1) - p[-1]) / (p[-1] - p[-2])
    W = Wf[:_P]
    W[_P - 1] += (1.0 + d) * Wf[_P]
    W[_P - 2] += (-d) * Wf[_P]
    return np.ascontiguousarray(W.astype(np.float32))


def _unique_cols(W):
    """Wu [256, U] = unique columns of W; inv [4096] with W = Wu[:, inv]."""
    uniq, inv = np.unique(W.T, axis=0, return_inverse=True)
    return np.ascontiguousarray(uniq.T), inv.astype(np.int64).ravel()


def _chunk_plan(W_nz, U):
    """Per <=512-col chunk of Wu: (col_lo, col_hi, k-halves with nonzeros).

    Full K=128 slices only: every matmul sits at PE tile_position (0, 0)
    and per-matmul fixed cost dominates, so zero sub-ranges are not
    split out. All-zero chunks get half 0 (zero W -> writes zeros).
    """
    nch = (U + _CH - 1) // _CH
    plan = []
    for c in range(nch):
        lo, hi = c * _CH, min(U, (c + 1) * _CH)
        halves = tuple(h for h in (0, 1)
                       if W_nz[128 * h:128 * (h + 1), lo:hi].any()) or (0,)
        plan.append((lo, hi, halves))
    return tuple(plan)


def _bf16_split(x):
    hi = x.astype(_BF16)
    lo = (x - hi.astype(np.float32)).astype(_BF16)
    return hi, lo


def _build_program(U, plan, use_wlo):
    from contextlib import ExitStack

    import concourse.bacc as bacc
    import concourse.bass as bass
    import concourse.mybir as mybir
    import concourse.tile as tile

    f32 = mybir.dt.float32
    bf16 = mybir.dt.bfloat16

    nc = bacc.Bacc("TRN2", target_bir_lowering=False, debug=False,
                   num_devices=_NC)
    # Transposed input, group-packed:
    # col = 1024*g + 512*comp + 256*half + 128*(tile in group) + batch.
    h_in = nc.dram_tensor("hxt", [128, 2 * 2 * _BS], bf16,
                          kind="ExternalInput").ap()
    # W halves side by side: wpk[r, half*U + c] = Wu[128*half + r, c].
    w_in = {"h": nc.dram_tensor("wh", [128, 2 * U], bf16,
                                kind="ExternalInput").ap()}
    if use_wlo:
        w_in["l"] = nc.dram_tensor("wl", [128, 2 * U], bf16,
                                   kind="ExternalInput").ap()
    # Tile-major output: batch tile bt at cols [bt*2U, (bt+1)*2U) so a
    # 2-tile store is one contiguous-row DMA (2KB rows at U=256).
    out = nc.dram_tensor("out", [128, _NBT * 2 * U], bf16,
                         kind="ExternalOutput").ap()

    # Pack (comp, chunk) accumulation groups into <=512-wide PSUM tiles.
    groups = [(x, lo, hi, hv) for x in (0, 1) for (lo, hi, hv) in plan]
    ps_specs, cur, curw = [], [], 0
    for g in groups:
        w = g[2] - g[1]
        if curw + w > _CH and cur:
            ps_specs.append((tuple(cur), curw))
            cur, curw = [], 0
        cur.append(g)
        curw += w
    ps_specs.append((tuple(cur), curw))

    with tile.TileContext(nc) as tc, ExitStack() as ctx:
        const_pool = ctx.enter_context(tc.tile_pool(name="const", bufs=1))
        out_pool = ctx.enter_context(tc.tile_pool(name="outp", bufs=3))
        ps_mm = ctx.enter_context(tc.tile_pool(name="psm", bufs=6,
                                               space="PSUM"))

        # W first on the sync ring (tiny, arrives before any output).
        w_sb = {}
        for part, wap in w_in.items():
            wt = const_pool.tile([128, 2 * U], bf16, tag=f"w{part}")
            nc.sync.dma_start(wt[:], wap[:, :])
            w_sb[part] = wt

        # Input group loads alternate the scalar/gpsimd rings (the only
        # HWDGE-capable engines besides sync, which carries the output).
        hx = []
        for g in range(_NG):
            t = const_pool.tile([128, _GT * 512], bf16, tag=f"hx{g}")
            eng = nc.scalar if g % 2 == 0 else nc.gpsimd
            eng.dma_start(t[:], h_in[:, g * _GT * 512:(g + 1) * _GT * 512])
            hx.append(t)

        terms = ["h"] if not use_wlo else ["h", "l"]
        cast_idx = 0
        ot = None
        for bt in range(_NBT):
            g, j = divmod(bt, _GT)
            if j == 0:  # one SBUF out tile per 2-tile store batch
                ot = out_pool.tile([128, _GT * 2 * U], bf16, tag="ot")
            off = j * 2 * U
            for (grs, wdt) in ps_specs:
                ps = ps_mm.tile([128, wdt], f32, tag="ps")
                poff = 0
                for (x, clo, chi, hv) in grs:
                    n_mm = len(hv) * len(terms)
                    k = 0
                    for h in hv:
                        for wp in terms:
                            nc.tensor.matmul(
                                ps[:, poff:poff + chi - clo],
                                hx[g][:, 512 * x + 256 * h + 128 * j:
                                      512 * x + 256 * h + 128 * j + 128],
                                w_sb[wp][:, h * U + clo:h * U + chi],
                                start=(k == 0),
                                stop=(k == n_mm - 1),
                            )
                            k += 1
                    poff += chi - clo
                dst = ot[:, off:off + wdt]
                if cast_idx % 2 == 0:
                    nc.vector.tensor_copy(dst, ps[:])
                else:
                    nc.scalar.copy(dst, ps[:])
                cast_idx += 1
                off += wdt
            if j == _GT - 1:
                eng = nc.sync if (bt // _GT) % 2 == 0 else nc.gpsimd
                eng.dma_start(
                    out[:, bass.ts(bt // _GT, _GT * 2 * U)], ot[:])

    nc.compile()
    return nc


def _get_program(U, plan, use_wlo):
    key = (U, plan, use_wlo)
    prog = _cache.get(key)
    if prog is None:
        prog = _build_program(U, plan, use_wlo)
        _cache[key] = prog
    return prog


def _prepare(H_real, H_imag, pilot_loc, alpha, beta):
    """Build (nc, in_maps, (U, inv)) for the spmd run."""
    H_real = np.ascontiguousarray(np.asarray(H_real, dtype=np.float32))
    H_imag = np.ascontiguousarray(np.asarray(H_imag, dtype=np.float32))
    pilot_loc = np.asarray(pilot_loc, dtype=np.float32)
    alpha = np.asarray(alpha, dtype=np.float32)
    beta = np.asarray(beta, dtype=np.float32)

    W = _interp_matrix(pilot_loc, alpha, beta)
    Wu, inv = _unique_cols(W)
    U = Wu.shape[1]
    wu_hi, wu_lo = _bf16_split(Wu)
    use_wlo = bool(np.any(np.asarray(wu_lo) != 0))
    plan = _chunk_plan(Wu != 0.0, U)
    nc = _get_program(U, plan, use_wlo)

    def pack_w(w):
        return np.ascontiguousarray(
            np.asarray(w).reshape(2, 128, U).transpose(1, 0, 2)
            .reshape(128, 2 * U))

    # Transposed bf16 inputs -> per-core group packing:
    # [comp, half, part, 16384] -> per core [part, g, comp, half, j, 128].
    X = np.stack([H_real.astype(_BF16).T, H_imag.astype(_BF16).T])
    X = np.ascontiguousarray(X)          # [2, 256, B]

    in_maps = []
    for i in range(_NC):
        slab = X[:, :, i * _BS:(i + 1) * _BS]          # [2, 256, BS]
        hxt = (slab.reshape(2, 2, 128, _NG, _GT, 128)  # x h part g j col
               .transpose(2, 3, 0, 1, 4, 5)            # part g x h j col
               .reshape(128, 4 * _BS))
        m = {"hxt": np.ascontiguousarray(hxt), "wh": pack_w(wu_hi)}
        if use_wlo:
            m["wl"] = pack_w(wu_lo)
        in_maps.append(m)
    return nc, in_maps, (U, inv)


def _assemble(results, U, inv):
    full = np.concatenate(
        [np.asarray(r["out"]).reshape(128, _NBT, 2 * U).transpose(1, 0, 2)
         .reshape(_BS, 2 * U) for r in results], axis=0)
    full = full.astype(np.float32)          # [B, 2U]: [real | imag]
    idx = np.empty(2 * _NFFT, np.int64)     # interleave (r, i) per t
    idx[0::2] = inv
    idx[1::2] = U + inv
    return full[:, idx].reshape(_B, _NFFT, 2)


def kernel(H_real, H_imag, pilot_loc, alpha, beta):
    nc, in_maps, (U, inv) = _prepare(H_real, H_imag, pilot_loc,
                                     alpha, beta)
    from concourse.bass_utils import run_bass_kernel_spmd

    res = run_bass_kernel_spmd(nc, in_maps, list(range(_NC))).results
    return _assemble(res, U, inv)


# revision 11
# speedup vs baseline: 1.3800x; 1.0751x over previous
"""Trainium2 Bass kernel for nn_Interpolator: pilot-to-subcarrier linear
interpolation with learned per-subcarrier weights.

Math: out[b, t] = alpha[t] * Hp[b, right[t]] + beta[t] * Hp[b, left[t]]
where Hp = [H, extrapolated last column]. The op is linear in H, so it
collapses to out = H @ W with a sparse W [256, 4096] built on the host
from (pilot_loc, alpha, beta); the extrapolation column folds into W's
last two rows.

Compression: output column t only depends on column t of W, so columns
of W that are identical produce identical output columns. The device
computes out_u = H @ Wu for the U *unique* columns of W only, and the
host scatters out_u's columns back to the full 4096 (pure indexing).
For this module's inputs (constant alpha/beta, stride-16 pilots) U=256,
a 16x cut in device output traffic. Falls back gracefully (same code
path) to any U up to 4096.

Precision: H and Wu are sent in bf16; out_u is stored in bf16. Error is
~2e-3 relative (bf16 rounding in, bf16 rounding out) against a 2e-2
gate. If Wu is not exactly bf16-representable a Wu_lo correction term
is added (for this module's inputs Wu is exact in bf16, so it is not).

Layout / schedule (from NTFF trace analysis):
- H^T is pre-transposed on the host and packed per core in 2-batch-tile
  "groups" [128, 2(comp) x 2(k-half) x 2(tile) x 128] so matmul lhsT
  slices come straight from SBUF with no on-chip transposes, and the
  8 group loads spread across 5 DMA rings (tensor/gpsimd/vector/
  scalar/sync) - the first matmul only waits for group 0 (~256KB).
- Whole-chunk matmuls: per batch tile, per comp, one N<=512 matmul per
  k-half accumulating into one PSUM bank (zero W regions contribute 0);
  per-matmul fixed cost (~150-200ns) dominates streaming, so fewer,
  wider matmuls win over skipping zero halves.
- One fp32->bf16 cast per PSUM tile, alternating DVE/ACT engines; out
  DMAs alternate sync/gpsimd rings so the write stream keeps pace with
  compute (single ring measured ~130GB/s at 1KB rows).

Sharding: data-parallel over the batch dim, 2048 rows per core x 8.
"""

import os
import sys

if os.path.isdir("/opt/trn_rl_repo") and "/opt/trn_rl_repo" not in sys.path:
    sys.path.insert(0, "/opt/trn_rl_repo")

import ml_dtypes
import numpy as np

_BF16 = np.dtype(ml_dtypes.bfloat16)

_B, _P, _NFFT = 16384, 256, 4096
_NC = 8
_BS = _B // _NC          # rows per core
_PT = 128                # partition tile (batch rows per tile)
_NBT = _BS // _PT        # batch tiles per core
_GT = 4                  # batch tiles per input-load group
_NG = _NBT // _GT        # input-load groups
_CH = 512                # max PSUM chunk width (one bank of fp32)

_cache = {}


def _interp_matrix(pilot_loc, alpha, beta):
    """W [256, 4096] f32 such that out = H @ W reproduces the reference."""
    p = pilot_loc.astype(np.float64) - 1.0  # reference: 1-based -> 0-based
    pp = np.concatenate([p, [float(_NFFT - 1)]])
    t = np.arange(_NFFT)
    left = np.clip(np.searchsorted(pp, t, side="right") - 1, 0, _P - 1)
    right = left + 1
    Wf = np.zeros((_P + 1, _NFFT), np.float64)
    Wf[left, t] += beta.astype(np.float64)
    Wf[right, t] += alpha.astype(np.float64)
    # Hp[:, P] = H[:, P-1] + slope * (NFFT-1 - p[-1]),
    # slope = (H[:, P-1] - H[:, P-2]) / (p[-1] - p[-2])  -> linear in H.
    d = (float(_NFFT - 1) - p[-1]) / (p[-1] - p[-2])
    W = Wf[:_P]
    W[_P - 1] += (1.0 + d) * Wf[_P]
    W[_P - 2] += (-d) * Wf[_P]
    return np.ascontiguousarray(W.astype(np.float32))


def _unique_cols(W):
    """Wu [256, U] = unique columns of W; inv [4096] with W = Wu[:, inv]."""
    uniq, inv = np.unique(W.T, axis=0, return_inverse=True)
    return np.ascontiguousarray(uniq.T), inv.astype(np.int64).ravel()


def _chunk_plan(W_nz, U):
    """Per <=512-col chunk of Wu: (col_lo, col_hi, k-halves with nonzeros).

    Full K=128 slices only: every matmul sits at PE tile_position (0, 0)
    and per-matmul fixed cost dominates, so zero sub-ranges are not
    split out. All-zero chunks get half 0 (zero W -> writes zeros).
    """
    nch = (U + _CH - 1) // _CH
    plan = []
    for c in range(nch):
        lo, hi = c * _CH, min(U, (c + 1) * _CH)
        halves = tuple(h for h in (0, 1)
                       if W_nz[128 * h:128 * (h + 1), lo:hi].any()) or (0,)
        plan.append((lo, hi, halves))
    return tuple(plan)


def _bf16_split(x):
    hi = x.astype(_BF16)
    lo = (x - hi.astype(np.float32)).astype(_BF16)
    return hi, lo


def _build_program(U, plan, use_wlo):
    from contextlib import ExitStack

    import concourse.bacc as bacc
    import concourse.bass as bass
    import concourse.mybir as mybir
    import concourse.tile as tile

    f32 = mybir.dt.float32
    bf16 = mybir.dt.bfloat16

    nc = bacc.Bacc("TRN2", target_bir_lowering=False, debug=False,
                   num_devices=_NC)
    # Transposed input, group-packed:
    # col = 1024*g + 512*comp + 256*half + 128*(tile in group) + batch.
    h_in = nc.dram_tensor("hxt", [128, 2 * 2 * _BS], bf16,
                          kind="ExternalInput").ap()
    # W halves side by side: wpk[r, half*U + c] = Wu[128*half + r, c].
    w_in = {"h": nc.dram_tensor("wh", [128, 2 * U], bf16,
                                kind="ExternalInput").ap()}
    if use_wlo:
        w_in["l"] = nc.dram_tensor("wl", [128, 2 * U], bf16,
                                   kind="ExternalInput").ap()
    # Tile-major output: batch tile bt at cols [bt*2U, (bt+1)*2U) so a
    # 2-tile store is one contiguous-row DMA (2KB rows at U=256).
    out = nc.dram_tensor("out", [128, _NBT * 2 * U], bf16,
                         kind="ExternalOutput").ap()

    # Pack (comp, chunk) accumulation groups into <=512-wide PSUM tiles.
    groups = [(x, lo, hi, hv) for x in (0, 1) for (lo, hi, hv) in plan]
    ps_specs, cur, curw = [], [], 0
    for g in groups:
        w = g[2] - g[1]
        if curw + w > _CH and cur:
            ps_specs.append((tuple(cur), curw))
            cur, curw = [], 0
        cur.append(g)
        curw += w
    ps_specs.append((tuple(cur), curw))

    # Output store batches: (start_tile, n_tiles, ring_idx). 4-tile
    # batches give 4KB DMA rows (~200GB/s/ring); the last 4 tiles split
    # 2+2 across two free rings so the drain tail is short.
    if _NBT >= 8:
        ob = [(s, 4) for s in range(0, _NBT - 4, 4)]
        ob += [(_NBT - 4, 2), (_NBT - 2, 2)]
    else:
        ob = [(s, min(4, _NBT - s)) for s in range(0, _NBT, 4)]
    ob_rings = [i % 2 for i in range(len(ob))]
    if len(ob_rings) >= 5:
        ob_rings[-1] = 2  # scalar: its input loads are done by then

    with tile.TileContext(nc) as tc, ExitStack() as ctx:
        const_pool = ctx.enter_context(tc.tile_pool(name="const", bufs=1))
        out_pool = ctx.enter_context(tc.tile_pool(name="outp",
                                                  bufs=len(ob)))
        ps_mm = ctx.enter_context(tc.tile_pool(name="psm", bufs=6,
                                               space="PSUM"))

        # W first on the sync ring (tiny, arrives before any output).
        w_sb = {}
        for part, wap in w_in.items():
            wt = const_pool.tile([128, 2 * U], bf16, tag=f"w{part}")
            nc.sync.dma_start(wt[:], wap[:, :])
            w_sb[part] = wt

        # Input group loads alternate the scalar/gpsimd rings (the only
        # HWDGE-capable engines besides sync, which carries the output).
        hx = []
        for g in range(_NG):
            t = const_pool.tile([128, _GT * 512], bf16, tag=f"hx{g}")
            eng = nc.scalar if g % 2 == 0 else nc.gpsimd
            eng.dma_start(t[:], h_in[:, g * _GT * 512:(g + 1) * _GT * 512])
            hx.append(t)

        terms = ["h"] if not use_wlo else ["h", "l"]
        out_rings = [nc.sync, nc.gpsimd, nc.scalar]
        cast_idx = 0
        for (s, n), ring in zip(ob, ob_rings):
            ot = out_pool.tile([128, n * 2 * U], bf16, tag=f"ot{n}")
            for j2 in range(n):
                bt = s + j2
                g, j = divmod(bt, _GT)
                off = j2 * 2 * U
                for (grs, wdt) in ps_specs:
                    ps = ps_mm.tile([128, wdt], f32, tag="ps")
                    poff = 0
                    for (x, clo, chi, hv) in grs:
                        n_mm = len(hv) * len(terms)
                        k = 0
                        for h in hv:
                            lo_ = (2 * _GT * 128) * x + (_GT * 128) * h \
                                + 128 * j
                            for wp in terms:
                                nc.tensor.matmul(
                                    ps[:, poff:poff + chi - clo],
                                    hx[g][:, lo_:lo_ + 128],
                                    w_sb[wp][:, h * U + clo:h * U + chi],
                                    start=(k == 0),
                                    stop=(k == n_mm - 1),
                                )
                                k += 1
                        poff += chi - clo
                    dst = ot[:, off:off + wdt]
                    if cast_idx % 2 == 0:
                        nc.vector.tensor_copy(dst, ps[:])
                    else:
                        nc.scalar.copy(dst, ps[:])
                    cast_idx += 1
                    off += wdt
            out_rings[ring].dma_start(
                out[:, s * 2 * U:(s + n) * 2 * U], ot[:])

    nc.compile()
    return nc


def _get_program(U, plan, use_wlo):
    key = (U, plan, use_wlo)
    prog = _cache.get(key)
    if prog is None:
        prog = _build_program(U, plan, use_wlo)
        _cache[key] = prog
    return prog


def _prepare(H_real, H_imag, pilot_loc, alpha, beta):
    """Build (nc, in_maps, (U, inv)) for the spmd run."""
    H_real = np.ascontiguousarray(np.asarray(H_real, dtype=np.float32))
    H_imag = np.ascontiguousarray(np.asarray(H_imag, dtype=np.float32))
    pilot_loc = np.asarray(pilot_loc, dtype=np.float32)
    alpha = np.asarray(alpha, dtype=np.float32)
    beta = np.asarray(beta, dtype=np.float32)

    W = _interp_matrix(pilot_loc, alpha, beta)
    Wu, inv = _unique_cols(W)
    U = Wu.shape[1]
    wu_hi, wu_lo = _bf16_split(Wu)
    use_wlo = bool(np.any(np.asarray(wu_lo) != 0))
    plan = _chunk_plan(Wu != 0.0, U)
    nc = _get_program(U, plan, use_wlo)

    def pack_w(w):
        return np.ascontiguousarray(
            np.asarray(w).reshape(2, 128, U).transpose(1, 0, 2)
            .reshape(128, 2 * U))

    # Transposed bf16 inputs -> per-core group packing:
    # [comp, half, part, 16384] -> per core [part, g, comp, half, j, 128].
    X = np.stack([H_real.astype(_BF16).T, H_imag.astype(_BF16).T])
    X = np.ascontiguousarray(X)          # [2, 256, B]

    in_maps = []
    for i in range(_NC):
        slab = X[:, :, i * _BS:(i + 1) * _BS]          # [2, 256, BS]
        hxt = (slab.reshape(2, 2, 128, _NG, _GT, 128)  # x h part g j col
               .transpose(2, 3, 0, 1, 4, 5)            # part g x h j col
               .reshape(128, 4 * _BS))
        m = {"hxt": np.ascontiguousarray(hxt), "wh": pack_w(wu_hi)}
        if use_wlo:
            m["wl"] = pack_w(wu_lo)
        in_maps.append(m)
    return nc, in_maps, (U, inv)


def _assemble(results, U, inv):
    full = np.concatenate(
        [np.asarray(r["out"]).reshape(128, _NBT, 2 * U).transpose(1, 0, 2)
         .reshape(_BS, 2 * U) for r in results], axis=0)
    full = full.astype(np.float32)          # [B, 2U]: [real | imag]
    idx = np.empty(2 * _NFFT, np.int64)     # interleave (r, i) per t
    idx[0::2] = inv
    idx[1::2] = U + inv
    return full[:, idx].reshape(_B, _NFFT, 2)


def kernel(H_real, H_imag, pilot_loc, alpha, beta):
    nc, in_maps, (U, inv) = _prepare(H_real, H_imag, pilot_loc,
                                     alpha, beta)
    from concourse.bass_utils import run_bass_kernel_spmd

    res = run_bass_kernel_spmd(nc, in_maps, list(range(_NC))).results
    return _assemble(res, U, inv)
